# revision 1
# baseline (speedup 1.0000x reference)
"""SincNet conv1d (KernelCustomFreezeConv) as a Bass/Tile TRN2 kernel.

Full inputs -> full output. Data-parallel over 8 NeuronCores: batch 32 is
sharded 4 per core; the 80x251 sinc filter bank is computed on host from the
three 80-scalar parameter vectors (tiny: ~20K flops) and replicated.

Per core the conv runs as matmuls on the tensor engine:
  out[f, t] = sum_k W[f,k] x[t+k],  k padded 251->256, split k = 128c+p.
  lhsT chunk c = W.T[128c:128c+128, :]  ([128, 80] stationary)
  rhs  chunk c = D[:, t0+128c : t0+128c+N] where D[p, m] = x[m+p] is a
  Toeplitz view materialized in SBUF by an overlapping-read DMA.
All chunks accumulate into one PSUM bank (fp32). For PE throughput, x and W
are each split hi/lo into two bf16 halves on host (x = xh + xl exactly to
~2^-17); the conv runs as 3 bf16 matmul passes per chunk (Wh.xh + Wh.xl +
Wl.xh, dropping the ~2^-18 Wl.xl term) at 1 cycle/row — 2.7x faster than the
4-cycle/row fp32 matmul path, at ~5e-6 relative error (vs 3e-6 all-fp32).
"""

import os
import numpy as np

FS = 16000.0
N_FILT = 80
FILT_DIM = 251
MIN_FREQ = 50.0

B_FULL = 32
L_IN = 32000
T_OUT = L_IN - FILT_DIM + 1  # 31750
N_CORES = 8
B_SHARD = B_FULL // N_CORES  # 4

KPAD = 256          # taps padded to 2*128
TILE_N = 512        # output cols per PSUM bank (fp32 max)
SEG_TILES = 8       # tiles per Toeplitz segment
SEG_COLS = TILE_N * SEG_TILES  # 4096
L_PAD = 32256       # padded x length so k in [251,256) reads zeros, not OOB

_cache = {}


def _build_filters(norm_f1, norm_f2, amplitude):
    """Mirror reference._build_filters in float32 numpy."""
    f32 = np.float32
    t_right = (np.linspace(1.0, (FILT_DIM - 1) / 2.0, (FILT_DIM - 1) // 2)
               .astype(f32) / f32(FS)).astype(f32)

    def sinc(band):
        arg = (2.0 * np.pi * band[:, None] * t_right[None, :]).astype(f32)
        y = (np.sin(arg) / arg).astype(f32)
        center = np.ones((band.shape[0], 1), dtype=f32)
        return np.concatenate([y[:, ::-1], center, y], axis=1)

    f1n = (np.abs(norm_f1) + f32(MIN_FREQ / FS)).astype(f32)
    f2n = (f1n + np.abs(norm_f2 - f1n) + f32(MIN_FREQ / FS)).astype(f32)
    f1 = (f1n * f32(FS)).astype(f32)
    f2 = (f2n * f32(FS)).astype(f32)
    amp = np.abs(amplitude).astype(f32)
    band = (amp[:, None] * (2.0 * f2[:, None] * sinc(f2)
                            - 2.0 * f1[:, None] * sinc(f1))).astype(f32)
    band = (band / band.max(axis=1, keepdims=True)).astype(f32)
    n = np.linspace(0.0, float(FILT_DIM), FILT_DIM).astype(f32)
    window = (0.54 - 0.46 * np.cos(2.0 * np.pi * n / FILT_DIM)).astype(f32)
    return (band * window[None, :]).astype(f32)  # [80, 251]


def _tiles_for(total):
    t0 = 0
    out = []
    while t0 < total:
        out.append((t0, min(TILE_N, total - t0)))
        t0 += TILE_N
    return out


def _build_program():
    import concourse.bacc as bacc
    import concourse.mybir as mybir
    from concourse import tile
    from concourse.ap import AP

    f32 = mybir.dt.float32
    bf16 = mybir.dt.bfloat16

    nc = bacc.Bacc("TRN2", target_bir_lowering=False, debug=False,
                   num_devices=N_CORES)
    # x hi/lo bf16 halves, interleaved as [2, B, L_PAD]: [0]=hi, [1]=lo
    x = nc.declare_dram_parameter("x", [2, B_SHARD, L_PAD], bf16,
                                  isOutput=False)
    # filters hi/lo: [2, KPAD, N_FILT]
    wt = nc.declare_dram_parameter("wt", [2, KPAD, N_FILT], bf16,
                                   isOutput=False)
    out = nc.declare_dram_parameter("out", [B_SHARD, N_FILT, T_OUT], f32,
                                    isOutput=True)

    with tile.TileContext(nc) as tc:
        with (
            tc.tile_pool(name="wpool", bufs=1) as wpool,
            tc.tile_pool(name="dpool", bufs=4) as dpool,
            tc.tile_pool(name="opool", bufs=4) as opool,
            tc.tile_pool(name="psum", bufs=8, space="PSUM") as psum_pool,
        ):
            w_sb = wpool.tile([128, 4 * N_FILT], bf16)
            # columns: [Wh0 | Wh1 | Wl0 | Wl1]
            for h in range(2):
                for c in range(2):
                    nc.sync.dma_start(
                        w_sb[:, (2 * h + c) * N_FILT:(2 * h + c + 1) * N_FILT],
                        wt[h][128 * c:128 * (c + 1), :])
            Wh = [w_sb[:, 0:N_FILT], w_sb[:, N_FILT:2 * N_FILT]]
            Wl = [w_sb[:, 2 * N_FILT:3 * N_FILT], w_sb[:, 3 * N_FILT:4 * N_FILT]]

            for b in range(B_SHARD):
                for s0 in range(0, T_OUT, SEG_COLS):
                    seg_cols = min(SEG_COLS, T_OUT - s0)
                    dw = seg_cols + 128
                    dh = dpool.tile([128, SEG_COLS + 128], bf16, tag="dhi")
                    dl = dpool.tile([128, SEG_COLS + 128], bf16, tag="dlo")
                    nc.sync.dma_start(
                        dh[:, :dw], AP(x, (0 * B_SHARD + b) * L_PAD + s0,
                                       [[1, 128], [1, dw]]))
                    nc.sync.dma_start(
                        dl[:, :dw], AP(x, (1 * B_SHARD + b) * L_PAD + s0,
                                       [[1, 128], [1, dw]]))
                    o_sb = opool.tile([128, SEG_COLS], f32, tag="oseg")
                    for (t0r, n) in _tiles_for(seg_cols):
                        ps = psum_pool.tile([128, TILE_N], f32)
                        hs = [dh[:, t0r:t0r + n], dh[:, t0r + 128:t0r + 128 + n]]
                        ls = [dl[:, t0r:t0r + n], dl[:, t0r + 128:t0r + 128 + n]]
                        # Wh.xh + Wh.xl + Wl.xh per chunk; Wl.xl dropped.
                        # Ordered so each stationary Wh chunk serves two
                        # consecutive matmuls.
                        nc.tensor.matmul(ps[:N_FILT, :n], Wh[0], hs[0],
                                         start=True, stop=False)
                        nc.tensor.matmul(ps[:N_FILT, :n], Wh[0], ls[0],
                                         start=False, stop=False)
                        nc.tensor.matmul(ps[:N_FILT, :n], Wh[1], hs[1],
                                         start=False, stop=False)
                        nc.tensor.matmul(ps[:N_FILT, :n], Wh[1], ls[1],
                                         start=False, stop=False)
                        nc.tensor.matmul(ps[:N_FILT, :n], Wl[0], hs[0],
                                         start=False, stop=False)
                        nc.tensor.matmul(ps[:N_FILT, :n], Wl[1], hs[1],
                                         start=False, stop=True)
                        nc.vector.tensor_copy(o_sb[:N_FILT, t0r:t0r + n],
                                              ps[:N_FILT, :n])
                    nc.scalar.dma_start(out[b][:, s0:s0 + seg_cols],
                                        o_sb[:N_FILT, :seg_cols])
    nc.finalize()
    return nc


def _get_program():
    if "nc" not in _cache:
        _cache["nc"] = _build_program()
    return _cache["nc"]


def kernel(x, norm_f1, norm_f2, amplitude, _trace=False):
    from concourse.bass_utils import run_bass_kernel_spmd

    x = np.asarray(x, dtype=np.float32)
    W = _build_filters(np.asarray(norm_f1, np.float32),
                       np.asarray(norm_f2, np.float32),
                       np.asarray(amplitude, np.float32))
    wt = np.zeros((KPAD, N_FILT), dtype=np.float32)
    wt[:FILT_DIM, :] = W.T

    import ml_dtypes
    bf = ml_dtypes.bfloat16
    wt_hi = wt.astype(bf)
    wt_lo = (wt - wt_hi.astype(np.float32)).astype(bf)
    wt2 = np.stack([wt_hi, wt_lo])  # [2, 256, 80]

    xs = x.reshape(B_FULL, L_IN)
    in_maps = []
    for c in range(N_CORES):
        shard = xs[c * B_SHARD:(c + 1) * B_SHARD]
        xp = np.zeros((B_SHARD, L_PAD), dtype=np.float32)
        xp[:, :L_IN] = shard
        x_hi = xp.astype(bf)
        x_lo = (xp - x_hi.astype(np.float32)).astype(bf)
        in_maps.append({"x": np.stack([x_hi, x_lo]), "wt": wt2})

    nc = _get_program()
    res = run_bass_kernel_spmd(nc, in_maps, list(range(N_CORES)))
    outs = [res.results[c]["out"] for c in range(N_CORES)]
    full = np.concatenate(outs, axis=0)  # [32, 80, 31750]
    if _trace:
        _cache["last_result"] = res
    return full



# revision 21
# speedup vs baseline: 2.7096x; 2.7096x over previous
"""SincNet conv1d (KernelCustomFreezeConv) as a Bass/Tile TRN2 kernel.

Full inputs -> full output. Data-parallel over 8 NeuronCores: batch 32 is
sharded 4 per core; the 80x251 sinc filter bank is computed on host from the
three 80-scalar parameter vectors (tiny: ~20K flops) and replicated.

Phase-decomposed conv (stride S=6): output time t = 6u + v. The matmul rhs
for phase v is ST[q, u] = x[6u + q] (q = tap row), which is a plain strided
view of a host de-interleaved x -- xd[r, j] = x[6j + r] -- so the Toeplitz
never needs materializing with 128x read amplification: SBUF tiles
T1 = ST[0:128], T2 = ST[128:256] are loaded with ~5x total amplification.
Taps k in [0, 251) are covered for every phase v in [0, 6) by two K=128
matmuls with phase-shifted zero-padded weights:
  chunk0: lhsT row p = W[:, p - v]        (rows p < v zero)
  chunk1: lhsT row p = W[:, 128 - v + p]  (rows p > 122 + v zero)
  out[f, 6u+v] = sum_p WA[v][p,f] T1[p,u] + sum_p WB[v][p,f] T2[p,u]

Precision: single bf16 pass (x and W rounded to bf16), bf16 output staged
via PSUM->SBUF copies round-robined over DVE/Act/Pool; rel err ~1e-3 vs the
2e-2 gate. Host reassembles out[b, f, 6u+v] = out_phased[b, v, f, u].
"""

import numpy as np

FS = 16000.0
N_FILT = 80
FILT_DIM = 251
MIN_FREQ = 50.0

B_FULL = 32
L_IN = 32000
T_OUT = L_IN - FILT_DIM + 1  # 31750
N_CORES = 8
B_SHARD = B_FULL // N_CORES  # 4

S = 6                 # phase stride
ROWLEN = 5376         # xd row length; 6*5376 = 32256 padded x
L_PAD = S * ROWLEN
U = 5292              # max columns per phase (ceil(31750/6))
TILE_N = 512

_cache = {}


def _build_filters(norm_f1, norm_f2, amplitude):
    """Mirror reference._build_filters in float32 numpy."""
    f32 = np.float32
    t_right = (np.linspace(1.0, (FILT_DIM - 1) / 2.0, (FILT_DIM - 1) // 2)
               .astype(f32) / f32(FS)).astype(f32)

    def sinc(band):
        arg = (2.0 * np.pi * band[:, None] * t_right[None, :]).astype(f32)
        y = (np.sin(arg) / arg).astype(f32)
        center = np.ones((band.shape[0], 1), dtype=f32)
        return np.concatenate([y[:, ::-1], center, y], axis=1)

    f1n = (np.abs(norm_f1) + f32(MIN_FREQ / FS)).astype(f32)
    f2n = (f1n + np.abs(norm_f2 - f1n) + f32(MIN_FREQ / FS)).astype(f32)
    f1 = (f1n * f32(FS)).astype(f32)
    f2 = (f2n * f32(FS)).astype(f32)
    amp = np.abs(amplitude).astype(f32)
    band = (amp[:, None] * (2.0 * f2[:, None] * sinc(f2)
                            - 2.0 * f1[:, None] * sinc(f1))).astype(f32)
    band = (band / band.max(axis=1, keepdims=True)).astype(f32)
    n = np.linspace(0.0, float(FILT_DIM), FILT_DIM).astype(f32)
    window = (0.54 - 0.46 * np.cos(2.0 * np.pi * n / FILT_DIM)).astype(f32)
    return (band * window[None, :]).astype(f32)  # [80, 251]


def _u_of(v):
    return (T_OUT - v + S - 1) // S  # outputs with t = S*u + v < T_OUT


def _tiles_for(total):
    t0, out = 0, []
    while t0 < total:
        out.append((t0, min(TILE_N, total - t0)))
        t0 += TILE_N
    return out


def _build_program():
    import concourse.bacc as bacc
    import concourse.mybir as mybir
    from concourse import tile
    from concourse.ap import AP

    f32 = mybir.dt.float32
    bf16 = mybir.dt.bfloat16

    nc = bacc.Bacc("TRN2", target_bir_lowering=False, debug=False,
                   num_devices=N_CORES)
    # xd: per batch, de-interleaved x: xd[b, r*ROWLEN + j] = x[b, S*j + r]
    x = nc.declare_dram_parameter("x", [B_SHARD * S * ROWLEN], bf16,
                                  isOutput=False)
    # 12 phase-shifted weight mats packed [128, 12*80] (SBUF layout)
    wt = nc.declare_dram_parameter("wt", [128, 2 * S * N_FILT], bf16,
                                   isOutput=False)
    # phased output: out[b, v, f, u]
    out = nc.declare_dram_parameter("out", [B_SHARD, S, N_FILT, U], bf16,
                                    isOutput=True)

    with tile.TileContext(nc) as tc:
        with (
            tc.tile_pool(name="wpool", bufs=1) as wpool,
            tc.tile_pool(name="dpool", bufs=2) as dpool,
            tc.tile_pool(name="opool", bufs=3) as opool,
            tc.tile_pool(name="psum", bufs=1, space="PSUM") as psum_pool,
        ):
            w_sb = wpool.tile([128, 2 * S * N_FILT], bf16)
            nc.sync.dma_start(w_sb, wt[:, :])
            WA = [w_sb[:, v * N_FILT:(v + 1) * N_FILT] for v in range(S)]
            WB = [w_sb[:, (S + v) * N_FILT:(S + v + 1) * N_FILT]
                  for v in range(S)]

            copy_engines = None  # set lazily below
            _g = [0]  # global psum slot counter (explicit 8-slot ring)

            def load_chunk(t1, t2, b, c0, n):
                # T1[p=6m+r, u] = x[6(u+m) + r];  T2[p, u] = ST[128+p, u]
                xb = b * S * ROWLEN
                nc.sync.dma_start(
                    t1[0:126, c0:c0 + n],
                    AP(x, xb + c0, [[1, 21], [ROWLEN, 6], [1, n]]))
                # p in [4,124): q = 128+p = 6*(22+m) + r
                nc.sync.dma_start(
                    t2[4:124, c0:c0 + n],
                    AP(x, xb + 22 + c0, [[1, 20], [ROWLEN, 6], [1, n]]))

            def load_small(t1, t2, b):
                # remainder partition rows, full width, once per batch
                xb = b * S * ROWLEN
                nc.sync.dma_start(
                    t1[126:128, :], AP(x, xb + 21, [[ROWLEN, 2], [1, U]]))
                # q = 128+p: p in [0,4): q = 6*21 + (2+p)
                nc.sync.dma_start(
                    t2[0:4, :],
                    AP(x, xb + 2 * ROWLEN + 21, [[ROWLEN, 4], [1, U]]))
                # p in [124,128): q = 6*42 + p-124
                nc.sync.dma_start(
                    t2[124:128, :], AP(x, xb + 42, [[ROWLEN, 4], [1, U]]))

            # batch 0 loads up-front in 4 column chunks (first matmuls can
            # start early); later batches stream 1/6-chunks between phases
            CW0 = (U + 3) // 4
            CWS = (U + S - 1) // S
            tiles_d = {0: (dpool.tile([128, U], bf16, tag="t1", name="t1_0"),
                           dpool.tile([128, U], bf16, tag="t2", name="t2_0"))}
            load_chunk(*tiles_d[0], 0, 0, 512)
            load_small(*tiles_d[0], 0)
            c0 = 512
            for n in (1024, 1756, U):  # graduated warm-up chunks
                n = min(n, U - c0)
                load_chunk(*tiles_d[0], 0, c0, n)
                c0 += n
                if c0 >= U:
                    break

            for b in range(B_SHARD):
                t1, t2 = tiles_d[b]
                if b + 1 < B_SHARD:
                    tiles_d[b + 1] = (
                        dpool.tile([128, U], bf16, tag="t1", name=f"t1_{b+1}"),
                        dpool.tile([128, U], bf16, tag="t2", name=f"t2_{b+1}"))

                for v in range(S):
                    uv = _u_of(v)
                    o_sb = opool.tile([128, U], bf16, tag="oseg")
                    if copy_engines is None:
                        copy_engines = [
                            lambda d, s: nc.vector.tensor_copy(d, s),
                            lambda d, s: nc.scalar.copy(d, s),
                        ]
                    for j, (c0, n) in enumerate(_tiles_for(uv)):
                        ps = psum_pool.tile([128, TILE_N], f32,
                                            tag=f"ps{(_g[0]) % 8}")
                        _g[0] += 1
                        nc.tensor.matmul(ps[:N_FILT, :n], WA[v],
                                         t1[:, c0:c0 + n],
                                         start=True, stop=False)
                        nc.tensor.matmul(ps[:N_FILT, :n], WB[v],
                                         t2[:, c0:c0 + n],
                                         start=False, stop=True)
                        copy_engines[j % 2](o_sb[:N_FILT, c0:c0 + n],
                                            ps[:N_FILT, :n])
                    # out DMAs on the Act queue (SP queue would serialize the
                    # next batch's input loads behind them), split so early
                    # pieces ship while the tail copies finish
                    nsplit = 4 if (b == B_SHARD - 1 and v == S - 1) else 2
                    hw_ = ((uv // nsplit) // TILE_N) * TILE_N
                    c = 0
                    for k in range(nsplit - 1):
                        nc.sync.dma_start(out[b][v][:, c:c + hw_],
                                          o_sb[:N_FILT, c:c + hw_])
                        c += hw_
                    nc.sync.dma_start(out[b][v][:, c:uv],
                                      o_sb[:N_FILT, c:uv])
                    # stream 1/6 of the next batch's input after each phase
                    # so input transfers pace with compute instead of
                    # bursting ahead of the output DMAs
                    if b + 1 < B_SHARD:
                        if v == 0:
                            load_small(*tiles_d[b + 1], b + 1)
                        nc0 = v * CWS
                        load_chunk(*tiles_d[b + 1], b + 1, nc0,
                                   min(CWS, U - nc0))
    nc.finalize()
    return nc


def _get_program():
    if "nc" not in _cache:
        _cache["nc"] = _build_program()
    return _cache["nc"]


def kernel(x, norm_f1, norm_f2, amplitude, _trace=False):
    from concourse.bass_utils import run_bass_kernel_spmd
    import ml_dtypes

    bf = ml_dtypes.bfloat16
    x = np.asarray(x, dtype=np.float32)
    W = _build_filters(np.asarray(norm_f1, np.float32),
                       np.asarray(norm_f2, np.float32),
                       np.asarray(amplitude, np.float32))
    Wb = W.astype(bf).astype(np.float32)  # rounded once, used per-shift

    # Phase-shifted zero-padded weights: WA[v][p] = W[:, p-v],
    # WB[v][p] = W[:, 128-v+p]
    wts = np.zeros((2 * S, 128, N_FILT), dtype=np.float32)
    for v in range(S):
        for p in range(128):
            k = p - v
            if 0 <= k < FILT_DIM:
                wts[v, p] = Wb[:, k]
            k = 128 - v + p
            if 0 <= k < FILT_DIM:
                wts[S + v, p] = Wb[:, k]
    # pack to the SBUF layout [128, 12*80]: column i*80+f = wts[i, p, f]
    wts = np.ascontiguousarray(
        wts.transpose(1, 0, 2).reshape(128, 2 * S * N_FILT)).astype(bf)

    xs = x.reshape(B_FULL, L_IN)
    in_maps = []
    for c in range(N_CORES):
        shard = xs[c * B_SHARD:(c + 1) * B_SHARD]
        xp = np.zeros((B_SHARD, L_PAD), dtype=np.float32)
        xp[:, :L_IN] = shard
        xpb = xp.astype(bf)
        # de-interleave: xd[b, r, j] = xp[b, S*j + r]
        xd = np.ascontiguousarray(
            xpb.reshape(B_SHARD, ROWLEN, S).transpose(0, 2, 1))
        in_maps.append({"x": xd.reshape(-1), "wt": wts})

    nc = _get_program()
    res = run_bass_kernel_spmd(nc, in_maps, list(range(N_CORES)))
    full = np.empty((B_FULL, N_FILT, T_OUT), dtype=np.float32)
    for c in range(N_CORES):
        op = np.asarray(res.results[c]["out"], dtype=np.float32)
        for v in range(S):
            full[c * B_SHARD:(c + 1) * B_SHARD, :, v::S] = \
                op[:, v, :, :_u_of(v)]
    if _trace:
        _cache["last_result"] = res
    return full


# revision 25
# speedup vs baseline: 2.8300x; 1.0444x over previous
"""SincNet conv1d (KernelCustomFreezeConv) as a Bass/Tile TRN2 kernel.

Full inputs -> full output. Data-parallel over 8 NeuronCores: batch 32 is
sharded 4 per core; the 80x251 sinc filter bank is computed on host from the
three 80-scalar parameter vectors (tiny: ~20K flops) and replicated.

Phase-decomposed conv (stride S=6): output time t = 6u + v. The matmul rhs
for phase v is ST[q, u] = x[6u + q] (q = tap row), which is a plain strided
view of a host de-interleaved x -- xd[r, j] = x[6j + r] -- so the Toeplitz
never needs materializing with 128x read amplification: SBUF tiles
T1 = ST[0:128], T2 = ST[128:256] are loaded with ~5x total amplification.
Taps k in [0, 251) are covered for every phase v in [0, 6) by two K=128
matmuls with phase-shifted zero-padded weights:
  chunk0: lhsT row p = W[:, p - v]        (rows p < v zero)
  chunk1: lhsT row p = W[:, 128 - v + p]  (rows p > 122 + v zero)
  out[f, 6u+v] = sum_p WA[v][p,f] T1[p,u] + sum_p WB[v][p,f] T2[p,u]

Precision: single bf16 pass (x and W rounded to bf16), bf16 output staged
via PSUM->SBUF copies round-robined over DVE/Act/Pool; rel err ~1e-3 vs the
2e-2 gate. Host reassembles out[b, f, 6u+v] = out_phased[b, v, f, u].
"""

import numpy as np

FS = 16000.0
N_FILT = 80
FILT_DIM = 251
MIN_FREQ = 50.0

B_FULL = 32
L_IN = 32000
T_OUT = L_IN - FILT_DIM + 1  # 31750
N_CORES = 8
B_SHARD = B_FULL // N_CORES  # 4

S = 6                 # phase stride
ROWLEN = 5376         # xd row length; 6*5376 = 32256 padded x
L_PAD = S * ROWLEN
U = 5292              # max columns per phase (ceil(31750/6))
TILE_N = 512

_cache = {}


def _build_filters(norm_f1, norm_f2, amplitude):
    """Mirror reference._build_filters in float32 numpy."""
    f32 = np.float32
    t_right = (np.linspace(1.0, (FILT_DIM - 1) / 2.0, (FILT_DIM - 1) // 2)
               .astype(f32) / f32(FS)).astype(f32)

    def sinc(band):
        arg = (2.0 * np.pi * band[:, None] * t_right[None, :]).astype(f32)
        y = (np.sin(arg) / arg).astype(f32)
        center = np.ones((band.shape[0], 1), dtype=f32)
        return np.concatenate([y[:, ::-1], center, y], axis=1)

    f1n = (np.abs(norm_f1) + f32(MIN_FREQ / FS)).astype(f32)
    f2n = (f1n + np.abs(norm_f2 - f1n) + f32(MIN_FREQ / FS)).astype(f32)
    f1 = (f1n * f32(FS)).astype(f32)
    f2 = (f2n * f32(FS)).astype(f32)
    amp = np.abs(amplitude).astype(f32)
    band = (amp[:, None] * (2.0 * f2[:, None] * sinc(f2)
                            - 2.0 * f1[:, None] * sinc(f1))).astype(f32)
    band = (band / band.max(axis=1, keepdims=True)).astype(f32)
    n = np.linspace(0.0, float(FILT_DIM), FILT_DIM).astype(f32)
    window = (0.54 - 0.46 * np.cos(2.0 * np.pi * n / FILT_DIM)).astype(f32)
    return (band * window[None, :]).astype(f32)  # [80, 251]


def _u_of(v):
    return (T_OUT - v + S - 1) // S  # outputs with t = S*u + v < T_OUT


def _tiles_for(total):
    t0, out = 0, []
    while t0 < total:
        out.append((t0, min(TILE_N, total - t0)))
        t0 += TILE_N
    return out


def _build_program():
    import concourse.bacc as bacc
    import concourse.mybir as mybir
    from concourse import tile
    from concourse.ap import AP

    f32 = mybir.dt.float32
    bf16 = mybir.dt.bfloat16

    nc = bacc.Bacc("TRN2", target_bir_lowering=False, debug=False,
                   num_devices=N_CORES)
    # xd: per batch, de-interleaved x: xd[b, r*ROWLEN + j] = x[b, S*j + r]
    x = nc.declare_dram_parameter("x", [B_SHARD * S * ROWLEN], bf16,
                                  isOutput=False)
    # 12 phase-shifted weight mats packed [128, 12*80] (SBUF layout)
    wt = nc.declare_dram_parameter("wt", [128, 2 * S * N_FILT], bf16,
                                   isOutput=False)
    # phased output: out[b, v, f, u]
    out = nc.declare_dram_parameter("out", [B_SHARD, S, N_FILT, U], bf16,
                                    isOutput=True)

    with tile.TileContext(nc) as tc:
        with (
            tc.tile_pool(name="wpool", bufs=1) as wpool,
            tc.tile_pool(name="dpool", bufs=2) as dpool,
            tc.tile_pool(name="opool", bufs=3) as opool,
            tc.tile_pool(name="psum", bufs=1, space="PSUM") as psum_pool,
        ):
            w_sb = wpool.tile([128, 2 * S * N_FILT], bf16)
            nc.sync.dma_start(w_sb, wt[:, :])
            WA = [w_sb[:, v * N_FILT:(v + 1) * N_FILT] for v in range(S)]
            WB = [w_sb[:, (S + v) * N_FILT:(S + v + 1) * N_FILT]
                  for v in range(S)]

            copy_engines = None  # set lazily below
            _g = [0]  # global psum slot counter (explicit 8-slot ring)

            def load_chunk(t1, t2, b, c0, n):
                # T1[p=6m+r, u] = x[6(u+m) + r];  T2[p, u] = ST[128+p, u]
                xb = b * S * ROWLEN
                nc.sync.dma_start(
                    t1[0:126, c0:c0 + n],
                    AP(x, xb + c0, [[1, 21], [ROWLEN, 6], [1, n]]))
                # p in [4,124): q = 128+p = 6*(22+m) + r
                nc.sync.dma_start(
                    t2[4:124, c0:c0 + n],
                    AP(x, xb + 22 + c0, [[1, 20], [ROWLEN, 6], [1, n]]))

            def load_small(t1, t2, b):
                # remainder partition rows, full width, once per batch
                xb = b * S * ROWLEN
                nc.sync.dma_start(
                    t1[126:128, :], AP(x, xb + 21, [[ROWLEN, 2], [1, U]]))
                # q = 128+p: p in [0,4): q = 6*21 + (2+p)
                nc.sync.dma_start(
                    t2[0:4, :],
                    AP(x, xb + 2 * ROWLEN + 21, [[ROWLEN, 4], [1, U]]))
                # p in [124,128): q = 6*42 + p-124
                nc.sync.dma_start(
                    t2[124:128, :], AP(x, xb + 42, [[ROWLEN, 4], [1, U]]))

            # batch 0 loads up-front in 4 column chunks (first matmuls can
            # start early); later batches stream 1/6-chunks between phases
            CW0 = (U + 3) // 4
            CWS = (U + S - 1) // S
            tiles_d = {0: (dpool.tile([128, U], bf16, tag="t1", name="t1_0"),
                           dpool.tile([128, U], bf16, tag="t2", name="t2_0"))}
            load_chunk(*tiles_d[0], 0, 0, 512)
            load_small(*tiles_d[0], 0)
            for c0 in range(512, U, CW0):
                load_chunk(*tiles_d[0], 0, c0, min(CW0, U - c0))

            for b in range(B_SHARD):
                t1, t2 = tiles_d[b]
                if b + 1 < B_SHARD:
                    tiles_d[b + 1] = (
                        dpool.tile([128, U], bf16, tag="t1", name=f"t1_{b+1}"),
                        dpool.tile([128, U], bf16, tag="t2", name=f"t2_{b+1}"))

                for v in range(S):
                    uv = _u_of(v)
                    if b + 1 < B_SHARD:
                        if v == 0:
                            load_small(*tiles_d[b + 1], b + 1)
                            load_chunk(*tiles_d[b + 1], b + 1, 0, CWS)
                        if v < S - 1:
                            nc0 = (v + 1) * CWS
                            load_chunk(*tiles_d[b + 1], b + 1, nc0,
                                       min(CWS, U - nc0))
                    o_sb = opool.tile([128, U], bf16, tag="oseg")
                    if copy_engines is None:
                        copy_engines = [
                            lambda d, s: nc.vector.tensor_copy(d, s),
                            lambda d, s: nc.scalar.copy(d, s),
                        ]
                    for j, (c0, n) in enumerate(_tiles_for(uv)):
                        ps = psum_pool.tile([128, TILE_N], f32,
                                            tag=f"ps{(_g[0]) % 7}")
                        _g[0] += 1
                        nc.tensor.matmul(ps[:N_FILT, :n], WA[v],
                                         t1[:, c0:c0 + n],
                                         start=True, stop=False)
                        nc.tensor.matmul(ps[:N_FILT, :n], WB[v],
                                         t2[:, c0:c0 + n],
                                         start=False, stop=True)
                        copy_engines[j % 2](o_sb[:N_FILT, c0:c0 + n],
                                            ps[:N_FILT, :n])
                    # out DMAs on the Act queue (SP queue would serialize the
                    # next batch's input loads behind them), split so early
                    # pieces ship while the tail copies finish
                    nsplit = 4 if (b == B_SHARD - 1 and v == S - 1) else 1
                    hw_ = ((uv // nsplit) // TILE_N) * TILE_N
                    c = 0
                    for k in range(nsplit - 1):
                        nc.sync.dma_start(out[b][v][:, c:c + hw_],
                                          o_sb[:N_FILT, c:c + hw_])
                        c += hw_
                    nc.sync.dma_start(out[b][v][:, c:uv],
                                      o_sb[:N_FILT, c:uv])
                    # stream 1/6 of the next batch's input after each phase
                    # so input transfers pace with compute instead of
                    # bursting ahead of the output DMAs; the last chunk is
                    # hoisted before this phase's matmuls elsewhere

    nc.finalize()
    return nc


def _get_program():
    if "nc" not in _cache:
        _cache["nc"] = _build_program()
    return _cache["nc"]


def kernel(x, norm_f1, norm_f2, amplitude, _trace=False):
    from concourse.bass_utils import run_bass_kernel_spmd
    import ml_dtypes

    bf = ml_dtypes.bfloat16
    x = np.asarray(x, dtype=np.float32)
    W = _build_filters(np.asarray(norm_f1, np.float32),
                       np.asarray(norm_f2, np.float32),
                       np.asarray(amplitude, np.float32))
    Wb = W.astype(bf).astype(np.float32)  # rounded once, used per-shift

    # Phase-shifted zero-padded weights: WA[v][p] = W[:, p-v],
    # WB[v][p] = W[:, 128-v+p]
    wts = np.zeros((2 * S, 128, N_FILT), dtype=np.float32)
    for v in range(S):
        for p in range(128):
            k = p - v
            if 0 <= k < FILT_DIM:
                wts[v, p] = Wb[:, k]
            k = 128 - v + p
            if 0 <= k < FILT_DIM:
                wts[S + v, p] = Wb[:, k]
    # pack to the SBUF layout [128, 12*80]: column i*80+f = wts[i, p, f]
    wts = np.ascontiguousarray(
        wts.transpose(1, 0, 2).reshape(128, 2 * S * N_FILT)).astype(bf)

    xs = x.reshape(B_FULL, L_IN)
    in_maps = []
    for c in range(N_CORES):
        shard = xs[c * B_SHARD:(c + 1) * B_SHARD]
        xp = np.zeros((B_SHARD, L_PAD), dtype=np.float32)
        xp[:, :L_IN] = shard
        xpb = xp.astype(bf)
        # de-interleave: xd[b, r, j] = xp[b, S*j + r]
        xd = np.ascontiguousarray(
            xpb.reshape(B_SHARD, ROWLEN, S).transpose(0, 2, 1))
        in_maps.append({"x": xd.reshape(-1), "wt": wts})

    nc = _get_program()
    res = run_bass_kernel_spmd(nc, in_maps, list(range(N_CORES)))
    full = np.empty((B_FULL, N_FILT, T_OUT), dtype=np.float32)
    for c in range(N_CORES):
        op = np.asarray(res.results[c]["out"], dtype=np.float32)
        for v in range(S):
            full[c * B_SHARD:(c + 1) * B_SHARD, :, v::S] = \
                op[:, v, :, :_u_of(v)]
    if _trace:
        _cache["last_result"] = res
    return full

